# revision 26
# baseline (speedup 1.0000x reference)
"""CBTree bottom-up fold kernel for 8 trn2 NeuronCores.

Problem: complete 4-ary tree, 9 levels, 87381 nodes in BFS order, d=256.
  leaves (level 8): h = vectors[21845:]
  internal node:    h = tanh(sum_i W_i @ h_child_i + vectors[node])
  where W_i = lc[i]*Wl + rc[i]*Wr,  lc=[1,2/3,1/3,0], rc=[0,1/3,2/3,1].

Strategy (data-parallel over sibling groups):
  - Shard every level contiguously over 8 cores. Children of a core's
    parents are exactly the core's own previous-level outputs, so
    levels 7..2 run with zero communication.
  - One 8KB AllGather of the level-2 states (16 nodes), then every
    core redundantly folds levels 1..0 and writes the root.
  - On chip h lives transposed ([d, nodes], d split into two 128-row
    partition halves) so the tensor engine contracts over d. The
    host-side sharding step hands each core its slices already in this
    [d, nodes] layout (a zero-FLOP relayout done while slicing), so the
    device spends no PE/DVE cycles on transposes: level l is 8
    accumulating 128x128xN matmuls per output half (4 sibling
    positions x 2 d-halves), a 9th identity-weight matmul that adds
    the bias vector in PSUM, and a tanh on the scalar engine writing
    the next level's tile directly.
  - Default dtype is fp16 (fp32 PSUM accumulation): vs fp32r it halves
    every DMA stream and runs full-rate at any moving dim (fp32r drops
    to 1/4 rate below N=256), for ~3.4e-3 scale-relative error vs the
    fp32 reference (fp32r fallback: _mode="f32r", ~1e-3, slower).
  - The root would be an N=1 matmul (invalid ISA), so the last level
    computes 4 replicated copies of the root.
  - The root leaves the device in its transposed on-chip layout
    ([128, 2] fp16); the host transposes it back while unsharding.
"""

import numpy as np

F32 = None  # set on first _lazy_imports()

_BASS = {}


def _lazy_imports():
    global bass, bacc, mybir, tile, make_identity, run_bass_kernel_spmd, F32
    import concourse.bass as bass
    import concourse.mybir as mybir
    from concourse import bacc
    import concourse.tile as tile
    from concourse.masks import make_identity
    from concourse.bass_utils import run_bass_kernel_spmd
    F32 = mybir.dt.float32


N_CORES = 8
D = 256
B = 4
L = 9
SIZES = [B**l for l in range(L)]            # [1,4,16,64,256,1024,4096,16384,65536]
OFFSETS = np.concatenate([[0], np.cumsum(SIZES)])  # [0,1,5,21,85,341,1365,5461,21845,87381]
N_LEAF_CORE = SIZES[8] // N_CORES           # 8192
# local (sharded) levels produce parents at levels 7..2
LOC_LEVELS = [7, 6, 5, 4, 3, 2]
LOC_PAR = {l: SIZES[l] // N_CORES for l in LOC_LEVELS}   # 2048,512,128,32,8,2
N_VECS_LOC = sum(LOC_PAR.values())          # 2730
N_VECS_TAIL = int(OFFSETS[2]) + 4           # 5 real rows + 4x replicated root row


def _build_nc(mode="fp16", WARM=6, CHUNKS=None):
    if CHUNKS is None:
        CHUNKS = {7: 512, 6: 128, 5: 64, 4: 16}
    key = ("nc", mode, WARM, tuple(sorted(CHUNKS.items())))
    if key in _BASS:
        return _BASS[key]
    nc = bacc.Bacc(num_devices=N_CORES)
    mmdt = {"f32r": mybir.dt.float32r, "fp32": F32,
            "fp16": mybir.dt.float16}[mode]
    dsz = 2 if mode == "fp16" else 4

    # all h/vec tensors arrive transposed: [256 = 2x128 d-rows, n nodes]
    leavesT = nc.declare_dram_parameter("leavesT", [D, N_LEAF_CORE], mmdt, isOutput=False)
    vecs_locT = nc.declare_dram_parameter("vecs_locT", [D, N_VECS_LOC], mmdt, isOutput=False)
    vecs_tailT = nc.declare_dram_parameter("vecs_tailT", [D, N_VECS_TAIL], mmdt, isOutput=False)
    wmat = nc.declare_dram_parameter("wmat", [128, 16 * 128], mmdt, isOutput=False)
    # root in transposed layout: row k, col mh -> component mh*128+k
    out = nc.declare_dram_parameter("out", [128, 2], mmdt, isOutput=True)

    NLOC2 = SIZES[2] // N_CORES                        # 2

    with tile.TileContext(nc) as tc:
        with (
            tc.tile_pool(name="const", bufs=1) as const_pool,
            tc.tile_pool(name="hbuf", bufs=1) as hbuf,
            tc.tile_pool(name="vecp", bufs=8) as vec_pool,
            tc.tile_pool(name="pmm", bufs=6, space="PSUM") as psum_mm,
            tc.tile_pool(name="pwarm", bufs=2, space="PSUM") as psum_warm,
            tc.tile_pool(name="dram", bufs=1, space="DRAM") as dram_pool,
        ):
            ident = const_pool.tile([128, 128], mmdt if mode == "fp16" else F32,
                                    name="ident")
            make_identity(nc, ident)
            # touch Tanh once so Bacc's activation-table load happens during
            # the initial DMA shadow instead of before the first real tanh
            warm_act = const_pool.tile([128, 4], F32, name="warm_act")
            nc.scalar.activation(warm_act[:1, :4], ident[:1, :4],
                                 mybir.ActivationFunctionType.Tanh)

            # leaf states: one tile per kh half (separate tiles keep the
            # per-chunk DMA write regions disjoint for the dep tracker)
            hT8 = [hbuf.tile([128, N_LEAF_CORE], mmdt, name=f"hT8_{kh}",
                             tag=f"hT8_{kh}") for kh in (0, 1)]

            # persistent transposed h states, one tile per (level, d-half)
            def h_tiles(name, n):
                return [hbuf.tile([128, max(n, 1)], mmdt, name=f"{name}_{kh}", tag=f"{name}_{kh}")
                        for kh in (0, 1)]

            hT = {7: h_tiles("hT7", 2048), 6: h_tiles("hT6", 512),
                  5: h_tiles("hT5", 128), 4: h_tiles("hT4", 32),
                  3: h_tiles("hT3", 8)}
            # level-2 state and the gathered level-2 array live packed
            # (both d-halves in one tile) so the AG bounce is 1 DMA each way
            t2p = hbuf.tile([128, 2 * NLOC2], mmdt, name="hT2p", tag="hT2p")
            hT[2] = [t2p[:, 0:NLOC2], t2p[:, NLOC2:2 * NLOC2]]
            h2ap = hbuf.tile([128, 2 * SIZES[2]], mmdt, name="h2allp", tag="h2allp")
            # tail levels (replicated): level-1 and root, packed tiles
            t1p = hbuf.tile([128, 8], mmdt, name="hTt1p", tag="hTt1p")
            hTt1 = [t1p[:, 0:4], t1p[:, 4:8]]
            t0p = hbuf.tile([128, 8], mmdt, name="hTt0p", tag="hTt0p")
            hTt0 = [t0p[:, 0:4], t0p[:, 4:8]]

            wsb = const_pool.tile([128, 16 * 128], mmdt, name="wsb")
            vloc = vec_pool.tile([128, 2, N_VECS_LOC], mmdt, name="vloc",
                                 tag="vloc", bufs=1)
            vtail = vec_pool.tile([128, 2, N_VECS_TAIL], mmdt, name="vtail",
                                  tag="vtail", bufs=1)

            CH7 = 256                       # level-7 compute chunk (parents)
            NPC = LOC_PAR[7] // CH7         # 8 leaf/vloc pieces

            def leaf_piece(j):
                c0, c1 = 4 * CH7 * j, 4 * CH7 * (j + 1)
                for kh in (0, 1):
                    nc.sync.dma_start(
                        hT8[kh][:, c0:c1],
                        leavesT[kh * 128:(kh + 1) * 128, c0:c1])

            vloc_src = vecs_locT[:].rearrange("(mh k) n -> k mh n", mh=2)

            def vloc_cols(c0, c1):
                nc.scalar.dma_start(vloc[:, :, c0:c1], vloc_src[:, :, c0:c1])

            def load_vtail():
                nc.scalar.dma_start(
                    vtail[:],
                    vecs_tailT[:].rearrange("(mh k) n -> k mh n", mh=2))

            def pe_warm(n):
                # burn the PE p-state ramp on the identity tile while the
                # first leaf/weight DMAs stream in
                rhs = ident[:, :].unsqueeze(1).broadcast_to([128, 4, 128])
                for _ in range(n):
                    scr = psum_warm.tile([128, 512], F32, name="ps_w", tag="w")
                    nc.tensor.matmul(scr[:, :512], ident[:], rhs,
                                     start=True, stop=True)

            # ---- shared level routine ----
            def do_level(child, n_par, vec_tile, vec_col0, hT_out,
                         chunk_prologue=None, chunk=512, rview_fn=None,
                         chunk_list=None):
                if rview_fn is None:
                    rview = [child[kh][:, :4 * n_par].rearrange(
                        "k (p four) -> k p four", four=4) for kh in (0, 1)]

                    def rview_fn(kh, c0, N, i):
                        return rview[kh][:, c0:c0 + N, i]
                if chunk_list is None:
                    chunk_list = [(c0, min(chunk, n_par - c0))
                                  for c0 in range(0, n_par, chunk)]
                for idx, (c0, N) in enumerate(chunk_list):
                    if chunk_prologue is not None:
                        chunk_prologue(idx)
                    vts = [vec_tile[:, mh, vec_col0 + c0: vec_col0 + c0 + N]
                           for mh in (0, 1)]
                    for mh in (0, 1):
                        ps = psum_mm.tile([128, 512], F32, name="ps_mm", tag="mm")
                        for i in range(4):
                            for kh in (0, 1):
                                blk = mh * 8 + i * 2 + kh
                                w = wsb[:, blk * 128:(blk + 1) * 128]
                                nc.tensor.matmul(ps[:, :N], w,
                                                 rview_fn(kh, c0, N, i),
                                                 start=(i == 0 and kh == 0),
                                                 stop=False)
                        nc.tensor.matmul(ps[:, :N], ident[:],
                                         vts[mh][:, :N], start=False, stop=True)
                        nc.scalar.activation(hT_out[mh][:, c0:c0 + N], ps[:, :N],
                                             mybir.ActivationFunctionType.Tanh)

            # ---- all HBM traffic issued up-front in stream (consumption)
            # order; the DMA queues' flow control paces it. PE warms on
            # ident while the first pieces land.
            leaf_piece(0)
            nc.scalar.dma_start(wsb[:, :8 * 128], wmat[:, :8 * 128])
            leaf_piece(1)
            nc.scalar.dma_start(wsb[:, 8 * 128:], wmat[:, 8 * 128:])
            vloc_cols(0, CH7)
            vloc_cols(CH7, 2 * CH7)
            for j in range(2, NPC):
                leaf_piece(j)
                vloc_cols(j * CH7, (j + 1) * CH7)
                if j == 3:
                    # level-6 bias vecs, in time for the interleaved chunks
                    vloc_cols(2048, 2560)
            vloc_cols(2560, N_VECS_LOC)     # levels 5..2 bias vecs
            load_vtail()
            pe_warm(WARM)

            # ---- local levels 7..2, with level-6 chunks interleaved into
            # level 7's back half so only the small final-chunk cascade
            # follows the last leaf DMA
            vcol0 = {}
            acc = 0
            for l in LOC_LEVELS:
                vcol0[l] = acc
                acc += LOC_PAR[l]
            child_of = {l: (hT8 if l == 7 else hT[l + 1]) for l in LOC_LEVELS}
            rviews = {l: [child_of[l][kh][:, :4 * LOC_PAR[l]].rearrange(
                "k (p four) -> k p four", four=4) for kh in (0, 1)]
                for l in LOC_LEVELS}

            def emit_chunk(l, c0, N):
                vts = [vloc[:, mh, vcol0[l] + c0: vcol0[l] + c0 + N]
                       for mh in (0, 1)]
                for mh in (0, 1):
                    ps = psum_mm.tile([128, 512], F32, name="ps_mm", tag="mm")
                    for i in range(4):
                        for kh in (0, 1):
                            blk = mh * 8 + i * 2 + kh
                            nc.tensor.matmul(
                                ps[:, :N], wsb[:, blk * 128:(blk + 1) * 128],
                                rviews[l][kh][:, c0:c0 + N, i],
                                start=(i == 0 and kh == 0), stop=False)
                    nc.tensor.matmul(ps[:, :N], ident[:], vts[mh][:, :N],
                                     start=False, stop=True)
                    nc.scalar.activation(hT[l][mh][:, c0:c0 + N], ps[:, :N],
                                         mybir.ActivationFunctionType.Tanh)

            emit_order = [(7, c0, CH7) for c0 in range(0, 5 * CH7, CH7)]
            for j, c6 in zip(range(5, NPC), range(0, 512, 128)):
                emit_order += [(6, c6, 128), (7, j * CH7, CH7)]
            emit_order += [(6, 384, 128),
                           (5, 0, 64), (5, 64, 64),
                           (4, 0, 16), (4, 16, 16),
                           (3, 0, 8), (2, 0, 2)]
            for l, c0, N in emit_order:
                emit_chunk(l, c0, N)

            # ---- AllGather of level-2 states, transposed layout ----
            # per-rank bounce [256 d, 2 nodes]; gathered [8*256, 2]
            cc_in = dram_pool.tile([D, NLOC2], mmdt, name="cc_in")
            cc_out = dram_pool.tile([N_CORES * D, NLOC2], mmdt,
                                    name="cc_out")
            nc.sync.dma_start(
                cc_in[:].rearrange("(kh k) n -> k kh n", kh=2),
                t2p[:].rearrange("k (kh n) -> k kh n", kh=2))
            nc.gpsimd.collective_compute(
                "AllGather", mybir.AluOpType.bypass,
                replica_groups=[list(range(N_CORES))],
                ins=[cc_in.opt()], outs=[cc_out.opt()])
            # single return DMA: keep the gathered layout (r kh n)-major in
            # columns — h2ap col = r*4 + kh*2 + n, fully contiguous dst
            nc.sync.dma_start(
                h2ap[:].rearrange("k (rk n) -> k rk n", rk=2 * N_CORES),
                cc_out[:].rearrange("(rk k) n -> k rk n", rk=2 * N_CORES))

            # ---- replicated tail: level 1, then 4 copies of the root ----
            # level-2 node j=(r*2+n) at sibling pos i of parent p: j=4p+i,
            # h2ap col = 8p + 4*(i//2) + 2*kh + i%2
            h2v = h2ap[:].rearrange("k (p i2 khd i1) -> k p i2 khd i1",
                                    p=4, i2=2, khd=2)

            def rv_l1(kh, c0, N, i):
                return h2v[:, c0:c0 + N, i // 2, kh, i % 2]

            do_level(None, SIZES[1], vtail, 1, hTt1, rview_fn=rv_l1)

            def rv_root(kh, c0, N, i):
                return hTt1[kh][:, i:i + 1].broadcast_to([128, N])

            do_level(None, 4, vtail, int(OFFSETS[2]), hTt0,
                     rview_fn=rv_root)

            # root col 0 of each mh half, transposed layout; host undoes it
            nc.sync.dma_start(
                out[:],
                t0p[:].rearrange("k (mh c) -> k mh c", mh=2)[:, :, 0])

    nc.finalize()
    _BASS[key] = nc
    return nc


def _prep_inputs(vectors, Wl, Wr, mode="fp16"):
    vectors = np.asarray(vectors, dtype=np.float32)
    Wl = np.asarray(Wl, dtype=np.float32)
    Wr = np.asarray(Wr, dtype=np.float32)

    ind = np.arange(1, B + 1, dtype=np.float32)
    lc = (B - ind) / (B - 1)
    rc = (ind - 1) / (B - 1)
    # W_t[i] = W_i.T laid out [k', (mh, i, kh, m')] for SBUF [128, 2048]
    Wt = np.stack([lc[i] * Wl.T + rc[i] * Wr.T for i in range(B)])  # [4, 256k, 256m]
    W5 = Wt.reshape(4, 2, 128, 2, 128)            # [i, kh, k', mh, m']
    halves = [W5[:, :, :, mh, :].reshape(4, 2, 128, 128)
              .transpose(2, 0, 1, 3).reshape(128, 8 * 128) for mh in (0, 1)]
    wmat = np.ascontiguousarray(np.concatenate(halves, axis=1),
                                dtype=np.float32)

    # one transposed copy of the node array; all per-core slices are views
    # into it laid out [d, nodes] (part of sharding, no arithmetic)
    vecsT = np.ascontiguousarray(vectors.T)                      # [256, 87381]
    vecs_tailT = np.ascontiguousarray(
        np.concatenate([vecsT[:, :int(OFFSETS[2])],
                        np.repeat(vecsT[:, 0:1], 4, axis=1)], axis=1))
    import ml_dtypes  # noqa: F401  (fp16 path uses numpy's float16)
    hdt = np.float16 if mode == "fp16" else np.float32
    in_maps = []
    for c in range(N_CORES):
        o8 = int(OFFSETS[8])
        leavesT_c = vecsT[:, o8 + c * N_LEAF_CORE: o8 + (c + 1) * N_LEAF_CORE]
        loc_parts = []
        for l in LOC_LEVELS:
            npl = LOC_PAR[l]
            o = int(OFFSETS[l])
            loc_parts.append(vecsT[:, o + c * npl: o + (c + 1) * npl])
        im = {
            "leavesT": np.ascontiguousarray(leavesT_c).astype(hdt),
            "vecs_locT": np.ascontiguousarray(
                np.concatenate(loc_parts, axis=1)).astype(hdt),
            "vecs_tailT": vecs_tailT.astype(hdt),
            "wmat": wmat.astype(hdt),
        }
        in_maps.append(im)
    return in_maps


def kernel(vectors, Wl, Wr, branching, n_levels, _mode="fp16"):
    _lazy_imports()
    assert int(branching) == B and int(n_levels) == L
    vectors = np.asarray(vectors)
    assert vectors.shape == (int(OFFSETS[L]), D), vectors.shape

    nc = _build_nc(mode=_mode)
    in_maps = _prep_inputs(vectors, Wl, Wr, mode=_mode)
    try:
        res = run_bass_kernel_spmd(nc, in_maps, core_ids=list(range(N_CORES)),
                                   trace=False)
    except Exception:
        # transient device hiccups (e.g. NRT_EXEC_UNIT_UNRECOVERABLE right
        # after another process released the cores) clear on a retry
        res = run_bass_kernel_spmd(nc, in_maps, core_ids=list(range(N_CORES)),
                                   trace=False)
    root = res.results[0]["out"]
    # undo the on-chip transposed layout: [128 k, 2 mh] -> d = mh*128 + k
    return np.asarray(root).astype(np.float32).T.reshape(1, D)


# revision 28
# speedup vs baseline: 1.1753x; 1.1753x over previous
"""CBTree bottom-up fold kernel for 8 trn2 NeuronCores.

Problem: complete 4-ary tree, 9 levels, 87381 nodes in BFS order, d=256.
  leaves (level 8): h = vectors[21845:]
  internal node:    h = tanh(sum_i W_i @ h_child_i + vectors[node])
  where W_i = lc[i]*Wl + rc[i]*Wr,  lc=[1,2/3,1/3,0], rc=[0,1/3,2/3,1].

Strategy (data-parallel over sibling groups):
  - Shard every level contiguously over 8 cores. Children of a core's
    parents are exactly the core's own previous-level outputs, so
    levels 7..2 run with zero communication.
  - One 8KB AllGather of the level-2 states (16 nodes), then every
    core redundantly folds levels 1..0 and writes the root.
  - On chip h lives transposed ([d, nodes], d split into two 128-row
    partition halves) so the tensor engine contracts over d. The
    host-side sharding step hands each core its slices already in this
    [d, nodes] layout (a zero-FLOP relayout done while slicing), so the
    device spends no PE/DVE cycles on transposes: level l is 8
    accumulating 128x128xN matmuls per output half (4 sibling
    positions x 2 d-halves), a 9th identity-weight matmul that adds
    the bias vector in PSUM, and a tanh on the scalar engine writing
    the next level's tile directly.
  - Default dtype is fp16 (fp32 PSUM accumulation): vs fp32r it halves
    every DMA stream and runs full-rate at any moving dim (fp32r drops
    to 1/4 rate below N=256), for ~3.4e-3 scale-relative error vs the
    fp32 reference (fp32r fallback: _mode="f32r", ~1e-3, slower).
  - The root would be an N=1 matmul (invalid ISA), so the last level
    computes 4 replicated copies of the root.
  - The root leaves the device in its transposed on-chip layout
    ([128, 2] fp16); the host transposes it back while unsharding.
"""

import numpy as np

F32 = None  # set on first _lazy_imports()

_BASS = {}


def _lazy_imports():
    global bass, bacc, mybir, tile, make_identity, run_bass_kernel_spmd, F32
    import concourse.bass as bass
    import concourse.mybir as mybir
    from concourse import bacc
    import concourse.tile as tile
    from concourse.masks import make_identity
    from concourse.bass_utils import run_bass_kernel_spmd
    F32 = mybir.dt.float32


N_CORES = 8
D = 256
B = 4
L = 9
SIZES = [B**l for l in range(L)]            # [1,4,16,64,256,1024,4096,16384,65536]
OFFSETS = np.concatenate([[0], np.cumsum(SIZES)])  # [0,1,5,21,85,341,1365,5461,21845,87381]
N_LEAF_CORE = SIZES[8] // N_CORES           # 8192
# local (sharded) levels produce parents at levels 7..2
LOC_LEVELS = [7, 6, 5, 4, 3, 2]
LOC_PAR = {l: SIZES[l] // N_CORES for l in LOC_LEVELS}   # 2048,512,128,32,8,2
N_VECS_LOC = sum(LOC_PAR.values())          # 2730
N_VECS_TAIL = int(OFFSETS[2]) + 4           # 5 real rows + 4x replicated root row


def _build_nc(mode="fp16", WARM=6, CHUNKS=None):
    if CHUNKS is None:
        CHUNKS = {7: 512, 6: 128, 5: 64, 4: 16}
    key = ("nc", mode, WARM, tuple(sorted(CHUNKS.items())))
    if key in _BASS:
        return _BASS[key]
    nc = bacc.Bacc(num_devices=N_CORES)
    mmdt = {"f32r": mybir.dt.float32r, "fp32": F32,
            "fp16": mybir.dt.float16}[mode]
    dsz = 2 if mode == "fp16" else 4

    # all h/vec tensors arrive transposed: [256 = 2x128 d-rows, n nodes]
    leavesT = nc.declare_dram_parameter("leavesT", [D, N_LEAF_CORE], mmdt, isOutput=False)
    vecs_locT = nc.declare_dram_parameter("vecs_locT", [D, N_VECS_LOC], mmdt, isOutput=False)
    vecs_tailT = nc.declare_dram_parameter("vecs_tailT", [D, N_VECS_TAIL], mmdt, isOutput=False)
    wmat = nc.declare_dram_parameter("wmat", [128, 16 * 128], mmdt, isOutput=False)
    # root in transposed layout: row k, col mh -> component mh*128+k
    out = nc.declare_dram_parameter("out", [128, 2], mmdt, isOutput=True)

    NLOC2 = SIZES[2] // N_CORES                        # 2

    with tile.TileContext(nc) as tc:
        with (
            tc.tile_pool(name="const", bufs=1) as const_pool,
            tc.tile_pool(name="hbuf", bufs=1) as hbuf,
            tc.tile_pool(name="vecp", bufs=8) as vec_pool,
            tc.tile_pool(name="pmm", bufs=6, space="PSUM") as psum_mm,
            tc.tile_pool(name="pwarm", bufs=2, space="PSUM") as psum_warm,
            tc.tile_pool(name="dram", bufs=1, space="DRAM") as dram_pool,
        ):
            ident = const_pool.tile([128, 128], mmdt if mode == "fp16" else F32,
                                    name="ident")
            make_identity(nc, ident)
            # touch Tanh once so Bacc's activation-table load happens during
            # the initial DMA shadow instead of before the first real tanh
            warm_act = const_pool.tile([128, 4], F32, name="warm_act")
            nc.scalar.activation(warm_act[:1, :4], ident[:1, :4],
                                 mybir.ActivationFunctionType.Tanh)

            # leaf states: one tile per kh half (separate tiles keep the
            # per-chunk DMA write regions disjoint for the dep tracker)
            hT8 = [hbuf.tile([128, N_LEAF_CORE], mmdt, name=f"hT8_{kh}",
                             tag=f"hT8_{kh}") for kh in (0, 1)]

            # persistent transposed h states, one tile per (level, d-half)
            def h_tiles(name, n):
                return [hbuf.tile([128, max(n, 1)], mmdt, name=f"{name}_{kh}", tag=f"{name}_{kh}")
                        for kh in (0, 1)]

            hT = {7: h_tiles("hT7", 2048), 6: h_tiles("hT6", 512),
                  5: h_tiles("hT5", 128), 4: h_tiles("hT4", 32),
                  3: h_tiles("hT3", 8)}
            # level-2 state and the gathered level-2 array live packed
            # (both d-halves in one tile) so the AG bounce is 1 DMA each way
            t2p = hbuf.tile([128, 2 * NLOC2], mmdt, name="hT2p", tag="hT2p")
            hT[2] = [t2p[:, 0:NLOC2], t2p[:, NLOC2:2 * NLOC2]]
            h2ap = hbuf.tile([128, 2 * SIZES[2]], mmdt, name="h2allp", tag="h2allp")
            # tail levels (replicated): level-1 and root, packed tiles
            t1p = hbuf.tile([128, 8], mmdt, name="hTt1p", tag="hTt1p")
            hTt1 = [t1p[:, 0:4], t1p[:, 4:8]]
            t0p = hbuf.tile([128, 8], mmdt, name="hTt0p", tag="hTt0p")
            hTt0 = [t0p[:, 0:4], t0p[:, 4:8]]

            wsb = const_pool.tile([128, 16 * 128], mmdt, name="wsb")
            vloc = vec_pool.tile([128, 2, N_VECS_LOC], mmdt, name="vloc",
                                 tag="vloc", bufs=1)
            vtail = vec_pool.tile([128, 2, N_VECS_TAIL], mmdt, name="vtail",
                                  tag="vtail", bufs=1)

            CH7 = 256                       # level-7 compute chunk (parents)
            NPC = LOC_PAR[7] // CH7         # 8 leaf/vloc pieces

            def leaf_piece(j):
                c0, c1 = 4 * CH7 * j, 4 * CH7 * (j + 1)
                for kh in (0, 1):
                    nc.sync.dma_start(
                        hT8[kh][:, c0:c1],
                        leavesT[kh * 128:(kh + 1) * 128, c0:c1])

            vloc_src = vecs_locT[:].rearrange("(mh k) n -> k mh n", mh=2)

            def vloc_cols(c0, c1):
                nc.sync.dma_start(vloc[:, :, c0:c1], vloc_src[:, :, c0:c1])

            def load_vtail():
                nc.scalar.dma_start(
                    vtail[:],
                    vecs_tailT[:].rearrange("(mh k) n -> k mh n", mh=2))

            def pe_warm(n):
                # burn the PE p-state ramp on the identity tile while the
                # first leaf/weight DMAs stream in
                rhs = ident[:, :].unsqueeze(1).broadcast_to([128, 4, 128])
                for _ in range(n):
                    scr = psum_warm.tile([128, 512], F32, name="ps_w", tag="w")
                    nc.tensor.matmul(scr[:, :512], ident[:], rhs,
                                     start=True, stop=True)

            # ---- shared level routine ----
            def do_level(child, n_par, vec_tile, vec_col0, hT_out,
                         chunk_prologue=None, chunk=512, rview_fn=None,
                         chunk_list=None):
                if rview_fn is None:
                    rview = [child[kh][:, :4 * n_par].rearrange(
                        "k (p four) -> k p four", four=4) for kh in (0, 1)]

                    def rview_fn(kh, c0, N, i):
                        return rview[kh][:, c0:c0 + N, i]
                if chunk_list is None:
                    chunk_list = [(c0, min(chunk, n_par - c0))
                                  for c0 in range(0, n_par, chunk)]
                for idx, (c0, N) in enumerate(chunk_list):
                    if chunk_prologue is not None:
                        chunk_prologue(idx)
                    vts = [vec_tile[:, mh, vec_col0 + c0: vec_col0 + c0 + N]
                           for mh in (0, 1)]
                    for mh in (0, 1):
                        ps = psum_mm.tile([128, 512], F32, name="ps_mm", tag="mm")
                        for i in range(4):
                            for kh in (0, 1):
                                blk = mh * 8 + i * 2 + kh
                                w = wsb[:, blk * 128:(blk + 1) * 128]
                                nc.tensor.matmul(ps[:, :N], w,
                                                 rview_fn(kh, c0, N, i),
                                                 start=(i == 0 and kh == 0),
                                                 stop=False)
                        nc.tensor.matmul(ps[:, :N], ident[:],
                                         vts[mh][:, :N], start=False, stop=True)
                        nc.scalar.activation(hT_out[mh][:, c0:c0 + N], ps[:, :N],
                                             mybir.ActivationFunctionType.Tanh)

            # ---- all HBM traffic issued up-front in stream (consumption)
            # order; the DMA queues' flow control paces it. Leaves and bias
            # vecs share the SP queue so their relative order is exact; the
            # weights and vtail ride the Activation queue (issued before any
            # tanh so its SEQ never parks mid-stream). PE warms on ident
            # while the first pieces land.
            leaf_piece(0)
            nc.scalar.dma_start(wsb[:, :8 * 128], wmat[:, :8 * 128])
            vloc_cols(0, 2 * CH7)
            nc.scalar.dma_start(wsb[:, 8 * 128:], wmat[:, 8 * 128:])
            load_vtail()
            leaf_piece(1)
            leaf_piece(2)
            vloc_cols(2 * CH7, 4 * CH7)
            leaf_piece(3)
            vloc_cols(2048, 2560)           # level-6 bias vecs
            leaf_piece(4)
            vloc_cols(4 * CH7, 6 * CH7)
            leaf_piece(5)
            vloc_cols(6 * CH7, 8 * CH7)
            leaf_piece(6)
            vloc_cols(2560, N_VECS_LOC)     # levels 5..2 bias vecs
            leaf_piece(7)
            pe_warm(WARM)

            # ---- local levels 7..2, with level-6 chunks interleaved into
            # level 7's back half so only the small final-chunk cascade
            # follows the last leaf DMA
            vcol0 = {}
            acc = 0
            for l in LOC_LEVELS:
                vcol0[l] = acc
                acc += LOC_PAR[l]
            child_of = {l: (hT8 if l == 7 else hT[l + 1]) for l in LOC_LEVELS}
            rviews = {l: [child_of[l][kh][:, :4 * LOC_PAR[l]].rearrange(
                "k (p four) -> k p four", four=4) for kh in (0, 1)]
                for l in LOC_LEVELS}

            def emit_chunk(l, c0, N):
                vts = [vloc[:, mh, vcol0[l] + c0: vcol0[l] + c0 + N]
                       for mh in (0, 1)]
                for mh in (0, 1):
                    ps = psum_mm.tile([128, 512], F32, name="ps_mm", tag="mm")
                    for i in range(4):
                        for kh in (0, 1):
                            blk = mh * 8 + i * 2 + kh
                            nc.tensor.matmul(
                                ps[:, :N], wsb[:, blk * 128:(blk + 1) * 128],
                                rviews[l][kh][:, c0:c0 + N, i],
                                start=(i == 0 and kh == 0), stop=False)
                    nc.tensor.matmul(ps[:, :N], ident[:], vts[mh][:, :N],
                                     start=False, stop=True)
                    nc.scalar.activation(hT[l][mh][:, c0:c0 + N], ps[:, :N],
                                         mybir.ActivationFunctionType.Tanh)

            emit_order = [(7, c0, CH7) for c0 in range(0, 5 * CH7, CH7)]
            for j, c6 in zip(range(5, NPC), range(0, 512, 128)):
                emit_order += [(6, c6, 128), (7, j * CH7, CH7)]
            emit_order += [(6, 384, 128),
                           (5, 0, 64), (5, 64, 64),
                           (4, 0, 16), (4, 16, 16),
                           (3, 0, 8), (2, 0, 2)]
            for l, c0, N in emit_order:
                emit_chunk(l, c0, N)

            # ---- AllGather of level-2 states, transposed layout ----
            # per-rank bounce [256 d, 2 nodes]; gathered [8*256, 2]
            cc_in = dram_pool.tile([D, NLOC2], mmdt, name="cc_in")
            cc_out = dram_pool.tile([N_CORES * D, NLOC2], mmdt,
                                    name="cc_out")
            nc.sync.dma_start(
                cc_in[:].rearrange("(kh k) n -> k kh n", kh=2),
                t2p[:].rearrange("k (kh n) -> k kh n", kh=2))
            nc.gpsimd.collective_compute(
                "AllGather", mybir.AluOpType.bypass,
                replica_groups=[list(range(N_CORES))],
                ins=[cc_in.opt()], outs=[cc_out.opt()])
            # single return DMA: keep the gathered layout (r kh n)-major in
            # columns — h2ap col = r*4 + kh*2 + n, fully contiguous dst
            nc.sync.dma_start(
                h2ap[:].rearrange("k (rk n) -> k rk n", rk=2 * N_CORES),
                cc_out[:].rearrange("(rk k) n -> k rk n", rk=2 * N_CORES))

            # ---- replicated tail: level 1, then 4 copies of the root ----
            # level-2 node j=(r*2+n) at sibling pos i of parent p: j=4p+i,
            # h2ap col = 8p + 4*(i//2) + 2*kh + i%2
            h2v = h2ap[:].rearrange("k (p i2 khd i1) -> k p i2 khd i1",
                                    p=4, i2=2, khd=2)

            def rv_l1(kh, c0, N, i):
                return h2v[:, c0:c0 + N, i // 2, kh, i % 2]

            do_level(None, SIZES[1], vtail, 1, hTt1, rview_fn=rv_l1)

            def rv_root(kh, c0, N, i):
                return hTt1[kh][:, i:i + 1].broadcast_to([128, N])

            do_level(None, 4, vtail, int(OFFSETS[2]), hTt0,
                     rview_fn=rv_root)

            # root col 0 of each mh half, transposed layout; host undoes it
            nc.sync.dma_start(
                out[:],
                t0p[:].rearrange("k (mh c) -> k mh c", mh=2)[:, :, 0])

    nc.finalize()
    _BASS[key] = nc
    return nc


def _prep_inputs(vectors, Wl, Wr, mode="fp16"):
    vectors = np.asarray(vectors, dtype=np.float32)
    Wl = np.asarray(Wl, dtype=np.float32)
    Wr = np.asarray(Wr, dtype=np.float32)

    ind = np.arange(1, B + 1, dtype=np.float32)
    lc = (B - ind) / (B - 1)
    rc = (ind - 1) / (B - 1)
    # W_t[i] = W_i.T laid out [k', (mh, i, kh, m')] for SBUF [128, 2048]
    Wt = np.stack([lc[i] * Wl.T + rc[i] * Wr.T for i in range(B)])  # [4, 256k, 256m]
    W5 = Wt.reshape(4, 2, 128, 2, 128)            # [i, kh, k', mh, m']
    halves = [W5[:, :, :, mh, :].reshape(4, 2, 128, 128)
              .transpose(2, 0, 1, 3).reshape(128, 8 * 128) for mh in (0, 1)]
    wmat = np.ascontiguousarray(np.concatenate(halves, axis=1),
                                dtype=np.float32)

    # one transposed copy of the node array; all per-core slices are views
    # into it laid out [d, nodes] (part of sharding, no arithmetic)
    vecsT = np.ascontiguousarray(vectors.T)                      # [256, 87381]
    vecs_tailT = np.ascontiguousarray(
        np.concatenate([vecsT[:, :int(OFFSETS[2])],
                        np.repeat(vecsT[:, 0:1], 4, axis=1)], axis=1))
    import ml_dtypes  # noqa: F401  (fp16 path uses numpy's float16)
    hdt = np.float16 if mode == "fp16" else np.float32
    in_maps = []
    for c in range(N_CORES):
        o8 = int(OFFSETS[8])
        leavesT_c = vecsT[:, o8 + c * N_LEAF_CORE: o8 + (c + 1) * N_LEAF_CORE]
        loc_parts = []
        for l in LOC_LEVELS:
            npl = LOC_PAR[l]
            o = int(OFFSETS[l])
            loc_parts.append(vecsT[:, o + c * npl: o + (c + 1) * npl])
        im = {
            "leavesT": np.ascontiguousarray(leavesT_c).astype(hdt),
            "vecs_locT": np.ascontiguousarray(
                np.concatenate(loc_parts, axis=1)).astype(hdt),
            "vecs_tailT": vecs_tailT.astype(hdt),
            "wmat": wmat.astype(hdt),
        }
        in_maps.append(im)
    return in_maps


def kernel(vectors, Wl, Wr, branching, n_levels, _mode="fp16"):
    _lazy_imports()
    assert int(branching) == B and int(n_levels) == L
    vectors = np.asarray(vectors)
    assert vectors.shape == (int(OFFSETS[L]), D), vectors.shape

    nc = _build_nc(mode=_mode)
    in_maps = _prep_inputs(vectors, Wl, Wr, mode=_mode)
    try:
        res = run_bass_kernel_spmd(nc, in_maps, core_ids=list(range(N_CORES)),
                                   trace=False)
    except Exception:
        # transient device hiccups (e.g. NRT_EXEC_UNIT_UNRECOVERABLE right
        # after another process released the cores) clear on a retry
        res = run_bass_kernel_spmd(nc, in_maps, core_ids=list(range(N_CORES)),
                                   trace=False)
    root = res.results[0]["out"]
    # undo the on-chip transposed layout: [128 k, 2 mh] -> d = mh*128 + k
    return np.asarray(root).astype(np.float32).T.reshape(1, D)


# revision 32
# speedup vs baseline: 1.1884x; 1.0112x over previous
"""CBTree bottom-up fold kernel for 8 trn2 NeuronCores.

Problem: complete 4-ary tree, 9 levels, 87381 nodes in BFS order, d=256.
  leaves (level 8): h = vectors[21845:]
  internal node:    h = tanh(sum_i W_i @ h_child_i + vectors[node])
  where W_i = lc[i]*Wl + rc[i]*Wr,  lc=[1,2/3,1/3,0], rc=[0,1/3,2/3,1].

Strategy (data-parallel over sibling groups):
  - Shard every level contiguously over 8 cores. Children of a core's
    parents are exactly the core's own previous-level outputs, so
    levels 7..2 run with zero communication.
  - One 8KB AllGather of the level-2 states (16 nodes), then every
    core redundantly folds levels 1..0 and writes the root.
  - On chip h lives transposed ([d, nodes], d split into two 128-row
    partition halves) so the tensor engine contracts over d. The
    host-side sharding step hands each core its slices already in this
    [d, nodes] layout (a zero-FLOP relayout done while slicing), so the
    device spends no PE/DVE cycles on transposes: level l is 8
    accumulating 128x128xN matmuls per output half (4 sibling
    positions x 2 d-halves), a 9th identity-weight matmul that adds
    the bias vector in PSUM, and a tanh on the scalar engine writing
    the next level's tile directly.
  - Default dtype is fp16 (fp32 PSUM accumulation): vs fp32r it halves
    every DMA stream and runs full-rate at any moving dim (fp32r drops
    to 1/4 rate below N=256), for ~3.4e-3 scale-relative error vs the
    fp32 reference (fp32r fallback: _mode="f32r", ~1e-3, slower).
  - The root would be an N=1 matmul (invalid ISA), so the last level
    computes 4 replicated copies of the root.
  - The root leaves the device in its transposed on-chip layout
    ([128, 2] fp16); the host transposes it back while unsharding.
"""

import numpy as np

F32 = None  # set on first _lazy_imports()

_BASS = {}


def _lazy_imports():
    global bass, bacc, mybir, tile, make_identity, run_bass_kernel_spmd, F32
    import concourse.bass as bass
    import concourse.mybir as mybir
    from concourse import bacc
    import concourse.tile as tile
    from concourse.masks import make_identity
    from concourse.bass_utils import run_bass_kernel_spmd
    F32 = mybir.dt.float32


N_CORES = 8
D = 256
B = 4
L = 9
SIZES = [B**l for l in range(L)]            # [1,4,16,64,256,1024,4096,16384,65536]
OFFSETS = np.concatenate([[0], np.cumsum(SIZES)])  # [0,1,5,21,85,341,1365,5461,21845,87381]
N_LEAF_CORE = SIZES[8] // N_CORES           # 8192
# local (sharded) levels produce parents at levels 7..2
LOC_LEVELS = [7, 6, 5, 4, 3, 2]
LOC_PAR = {l: SIZES[l] // N_CORES for l in LOC_LEVELS}   # 2048,512,128,32,8,2
N_VECS_LOC = sum(LOC_PAR.values())          # 2730
N_VECS_TAIL = int(OFFSETS[2]) + 4           # 5 real rows + 4x replicated root row


def _build_nc(mode="fp16", WARM=6, CHUNKS=None):
    if CHUNKS is None:
        CHUNKS = {7: 512, 6: 128, 5: 64, 4: 16}
    key = ("nc", mode, WARM, tuple(sorted(CHUNKS.items())))
    if key in _BASS:
        return _BASS[key]
    nc = bacc.Bacc(num_devices=N_CORES)
    mmdt = {"f32r": mybir.dt.float32r, "fp32": F32,
            "fp16": mybir.dt.float16}[mode]
    dsz = 2 if mode == "fp16" else 4

    # all h/vec tensors arrive transposed: [256 = 2x128 d-rows, n nodes]
    leavesT = nc.declare_dram_parameter("leavesT", [D, N_LEAF_CORE], mmdt, isOutput=False)
    vecs_locT = nc.declare_dram_parameter("vecs_locT", [D, N_VECS_LOC], mmdt, isOutput=False)
    vecs_tailT = nc.declare_dram_parameter("vecs_tailT", [D, N_VECS_TAIL], mmdt, isOutput=False)
    wmat = nc.declare_dram_parameter("wmat", [128, 16 * 128], mmdt, isOutput=False)
    # root in transposed layout: row k, col mh -> component mh*128+k
    out = nc.declare_dram_parameter("out", [128, 2], mmdt, isOutput=True)

    NLOC2 = SIZES[2] // N_CORES                        # 2

    with tile.TileContext(nc) as tc:
        with (
            tc.tile_pool(name="const", bufs=1) as const_pool,
            tc.tile_pool(name="hbuf", bufs=1) as hbuf,
            tc.tile_pool(name="vecp", bufs=8) as vec_pool,
            tc.tile_pool(name="pmm", bufs=6, space="PSUM") as psum_mm,
            tc.tile_pool(name="pwarm", bufs=2, space="PSUM") as psum_warm,
            tc.tile_pool(name="dram", bufs=1, space="DRAM") as dram_pool,
        ):
            ident = const_pool.tile([128, 128], mmdt if mode == "fp16" else F32,
                                    name="ident")
            make_identity(nc, ident)
            # touch Tanh once so Bacc's activation-table load happens during
            # the initial DMA shadow instead of before the first real tanh
            warm_act = const_pool.tile([128, 4], F32, name="warm_act")
            nc.scalar.activation(warm_act[:1, :4], ident[:1, :4],
                                 mybir.ActivationFunctionType.Tanh)

            # leaf states: one tile per kh half (separate tiles keep the
            # per-chunk DMA write regions disjoint for the dep tracker)
            hT8 = [hbuf.tile([128, N_LEAF_CORE], mmdt, name=f"hT8_{kh}",
                             tag=f"hT8_{kh}") for kh in (0, 1)]

            # persistent transposed h states, one tile per (level, d-half)
            def h_tiles(name, n):
                return [hbuf.tile([128, max(n, 1)], mmdt, name=f"{name}_{kh}", tag=f"{name}_{kh}")
                        for kh in (0, 1)]

            # small levels store both d-halves in one packed tile so one
            # activation (over a single PSUM bank) writes the whole level
            def h_packed(name, n):
                t = hbuf.tile([128, 2 * n], mmdt, name=name, tag=name)
                return t, [t[:, 0:n], t[:, n:2 * n]]

            hT = {7: h_tiles("hT7", 2048), 6: h_tiles("hT6", 512)}
            hTp = {}
            for l, nm in ((5, "hT5p"), (4, "hT4p"), (3, "hT3p")):
                hTp[l], hT[l] = h_packed(nm, LOC_PAR[l])
            # the level-2 packed tile doubles as the AG bounce source
            t2p, hT[2] = h_packed("hT2p", NLOC2)
            hTp[2] = t2p
            h2ap = hbuf.tile([128, 2 * SIZES[2]], mmdt, name="h2allp", tag="h2allp")
            # tail levels (replicated): level-1 and root, packed tiles
            t1p = hbuf.tile([128, 8], mmdt, name="hTt1p", tag="hTt1p")
            hTt1 = [t1p[:, 0:4], t1p[:, 4:8]]
            t0p = hbuf.tile([128, 8], mmdt, name="hTt0p", tag="hTt0p")
            hTt0 = [t0p[:, 0:4], t0p[:, 4:8]]

            wsb = const_pool.tile([128, 16 * 128], mmdt, name="wsb")
            vloc = vec_pool.tile([128, 2, N_VECS_LOC], mmdt, name="vloc",
                                 tag="vloc", bufs=1)
            vtail = vec_pool.tile([128, 2, N_VECS_TAIL], mmdt, name="vtail",
                                  tag="vtail", bufs=1)

            CH7 = 256                       # level-7 compute chunk (parents)
            NPC = LOC_PAR[7] // CH7         # 8 leaf/vloc pieces

            def leaf_piece(j):
                c0, c1 = 4 * CH7 * j, 4 * CH7 * (j + 1)
                for kh in (0, 1):
                    nc.sync.dma_start(
                        hT8[kh][:, c0:c1],
                        leavesT[kh * 128:(kh + 1) * 128, c0:c1])

            vloc_src = vecs_locT[:].rearrange("(mh k) n -> k mh n", mh=2)

            def vloc_cols(c0, c1):
                nc.sync.dma_start(vloc[:, :, c0:c1], vloc_src[:, :, c0:c1])

            def load_vtail():
                nc.scalar.dma_start(
                    vtail[:],
                    vecs_tailT[:].rearrange("(mh k) n -> k mh n", mh=2))

            def pe_warm(n):
                # burn the PE p-state ramp on the identity tile while the
                # first leaf/weight DMAs stream in
                rhs = ident[:, :].unsqueeze(1).broadcast_to([128, 4, 128])
                for _ in range(n):
                    scr = psum_warm.tile([128, 512], F32, name="ps_w", tag="w")
                    nc.tensor.matmul(scr[:, :512], ident[:], rhs,
                                     start=True, stop=True)

            # ---- tail level routine (single chunk, merged activation over
            # both d-halves of the packed output tile) ----
            def do_tail_level(n_par, vec_col0, packed_out, rview_fn):
                N = n_par
                vts = [vtail[:, mh, vec_col0: vec_col0 + N] for mh in (0, 1)]
                ps = psum_mm.tile([128, 512], F32, name="ps_mm", tag="mm")
                for mh in (0, 1):
                    pv = ps[:, mh * N:(mh + 1) * N]
                    for i in range(4):
                        for kh in (0, 1):
                            blk = mh * 8 + i * 2 + kh
                            nc.tensor.matmul(pv,
                                             wsb[:, blk * 128:(blk + 1) * 128],
                                             rview_fn(kh, 0, N, i),
                                             start=(i == 0 and kh == 0),
                                             stop=False)
                    nc.tensor.matmul(pv, ident[:], vts[mh][:, :N],
                                     start=False, stop=True)
                nc.scalar.activation(
                    packed_out[:].rearrange("k (kh n) -> k kh n", kh=2),
                    ps[:, :2 * N], mybir.ActivationFunctionType.Tanh)

            # ---- all HBM traffic issued up-front in stream (consumption)
            # order; the DMA queues' flow control paces it. Leaves and bias
            # vecs share the SP queue so their relative order is exact; the
            # weights and vtail ride the Activation queue (issued before any
            # tanh so its SEQ never parks mid-stream). PE warms on ident
            # while the first pieces land.
            leaf_piece(0)
            nc.scalar.dma_start(wsb[:, :8 * 128], wmat[:, :8 * 128])
            vloc_cols(0, 2 * CH7)
            nc.scalar.dma_start(wsb[:, 8 * 128:], wmat[:, 8 * 128:])
            load_vtail()
            leaf_piece(1)
            leaf_piece(2)
            vloc_cols(2 * CH7, 4 * CH7)
            leaf_piece(3)
            vloc_cols(2048, 2560)           # level-6 bias vecs
            leaf_piece(4)
            vloc_cols(4 * CH7, 6 * CH7)
            leaf_piece(5)
            vloc_cols(6 * CH7, 8 * CH7)
            leaf_piece(6)
            vloc_cols(2560, N_VECS_LOC)     # levels 5..2 bias vecs
            leaf_piece(7)
            pe_warm(WARM)

            # ---- local levels 7..2, with level-6 chunks interleaved into
            # level 7's back half so only the small final-chunk cascade
            # follows the last leaf DMA
            vcol0 = {}
            acc = 0
            for l in LOC_LEVELS:
                vcol0[l] = acc
                acc += LOC_PAR[l]
            child_of = {l: (hT8 if l == 7 else hT[l + 1]) for l in LOC_LEVELS}
            rviews = {l: [child_of[l][kh][:, :4 * LOC_PAR[l]].rearrange(
                "k (p four) -> k p four", four=4) for kh in (0, 1)]
                for l in LOC_LEVELS}

            def emit_chunk(l, c0, N):
                vts = [vloc[:, mh, vcol0[l] + c0: vcol0[l] + c0 + N]
                       for mh in (0, 1)]
                merged = l in hTp and 2 * N <= 512
                ps = (psum_mm.tile([128, 512], F32, name="ps_mm", tag="mm")
                      if merged else None)
                for mh in (0, 1):
                    if not merged:
                        ps = psum_mm.tile([128, 512], F32, name="ps_mm",
                                          tag="mm")
                    pv = ps[:, mh * N:(mh + 1) * N] if merged else ps[:, :N]
                    for i in range(4):
                        for kh in (0, 1):
                            blk = mh * 8 + i * 2 + kh
                            nc.tensor.matmul(
                                pv, wsb[:, blk * 128:(blk + 1) * 128],
                                rviews[l][kh][:, c0:c0 + N, i],
                                start=(i == 0 and kh == 0), stop=False)
                    nc.tensor.matmul(pv, ident[:], vts[mh][:, :N],
                                     start=False, stop=True)
                    if not merged:
                        nc.scalar.activation(hT[l][mh][:, c0:c0 + N],
                                             ps[:, :N],
                                             mybir.ActivationFunctionType.Tanh)
                if merged:
                    outv = hTp[l][:].rearrange(
                        "k (kh n) -> k kh n", kh=2)[:, :, c0:c0 + N]
                    nc.scalar.activation(outv, ps[:, :2 * N],
                                         mybir.ActivationFunctionType.Tanh)

            emit_order = [(7, c0, CH7) for c0 in range(0, 5 * CH7, CH7)]
            for j, c6 in zip(range(5, NPC), range(0, 512, 128)):
                emit_order += [(6, c6, 128), (7, j * CH7, CH7)]
            emit_order += [(6, 384, 128),
                           (5, 0, 64), (5, 64, 64),
                           (4, 0, 16), (4, 16, 16),
                           (3, 0, 8), (2, 0, 2)]
            for l, c0, N in emit_order:
                emit_chunk(l, c0, N)

            # ---- AllGather of level-2 states, transposed layout ----
            # per-rank bounce [256 d, 2 nodes]; gathered [8*256, 2]
            cc_in = dram_pool.tile([D, NLOC2], mmdt, name="cc_in")
            cc_out = dram_pool.tile([N_CORES * D, NLOC2], mmdt,
                                    name="cc_out")
            nc.sync.dma_start(
                cc_in[:].rearrange("(kh k) n -> k kh n", kh=2),
                t2p[:].rearrange("k (kh n) -> k kh n", kh=2))
            nc.gpsimd.collective_compute(
                "AllGather", mybir.AluOpType.bypass,
                replica_groups=[list(range(N_CORES))],
                ins=[cc_in.opt()], outs=[cc_out.opt()])
            # single return DMA: keep the gathered layout (r kh n)-major in
            # columns — h2ap col = r*4 + kh*2 + n, fully contiguous dst
            nc.sync.dma_start(
                h2ap[:].rearrange("k (rk n) -> k rk n", rk=2 * N_CORES),
                cc_out[:].rearrange("(rk k) n -> k rk n", rk=2 * N_CORES))

            # ---- replicated tail: level 1, then 4 copies of the root ----
            # level-2 node j=(r*2+n) at sibling pos i of parent p: j=4p+i,
            # h2ap col = 8p + 4*(i//2) + 2*kh + i%2
            h2v = h2ap[:].rearrange("k (p i2 khd i1) -> k p i2 khd i1",
                                    p=4, i2=2, khd=2)

            def rv_l1(kh, c0, N, i):
                return h2v[:, c0:c0 + N, i // 2, kh, i % 2]

            do_tail_level(SIZES[1], 1, t1p, rv_l1)

            def rv_root(kh, c0, N, i):
                return hTt1[kh][:, i:i + 1].broadcast_to([128, N])

            do_tail_level(4, int(OFFSETS[2]), t0p, rv_root)

            # root col 0 of each mh half, transposed layout; host undoes it
            nc.sync.dma_start(
                out[:],
                t0p[:].rearrange("k (mh c) -> k mh c", mh=2)[:, :, 0])

    nc.finalize()
    _BASS[key] = nc
    return nc


def _prep_inputs(vectors, Wl, Wr, mode="fp16"):
    vectors = np.asarray(vectors, dtype=np.float32)
    Wl = np.asarray(Wl, dtype=np.float32)
    Wr = np.asarray(Wr, dtype=np.float32)

    ind = np.arange(1, B + 1, dtype=np.float32)
    lc = (B - ind) / (B - 1)
    rc = (ind - 1) / (B - 1)
    # W_t[i] = W_i.T laid out [k', (mh, i, kh, m')] for SBUF [128, 2048]
    Wt = np.stack([lc[i] * Wl.T + rc[i] * Wr.T for i in range(B)])  # [4, 256k, 256m]
    W5 = Wt.reshape(4, 2, 128, 2, 128)            # [i, kh, k', mh, m']
    halves = [W5[:, :, :, mh, :].reshape(4, 2, 128, 128)
              .transpose(2, 0, 1, 3).reshape(128, 8 * 128) for mh in (0, 1)]
    wmat = np.ascontiguousarray(np.concatenate(halves, axis=1),
                                dtype=np.float32)

    # one transposed copy of the node array; all per-core slices are views
    # into it laid out [d, nodes] (part of sharding, no arithmetic)
    vecsT = np.ascontiguousarray(vectors.T)                      # [256, 87381]
    vecs_tailT = np.ascontiguousarray(
        np.concatenate([vecsT[:, :int(OFFSETS[2])],
                        np.repeat(vecsT[:, 0:1], 4, axis=1)], axis=1))
    import ml_dtypes  # noqa: F401  (fp16 path uses numpy's float16)
    hdt = np.float16 if mode == "fp16" else np.float32
    in_maps = []
    for c in range(N_CORES):
        o8 = int(OFFSETS[8])
        leavesT_c = vecsT[:, o8 + c * N_LEAF_CORE: o8 + (c + 1) * N_LEAF_CORE]
        loc_parts = []
        for l in LOC_LEVELS:
            npl = LOC_PAR[l]
            o = int(OFFSETS[l])
            loc_parts.append(vecsT[:, o + c * npl: o + (c + 1) * npl])
        im = {
            "leavesT": np.ascontiguousarray(leavesT_c).astype(hdt),
            "vecs_locT": np.ascontiguousarray(
                np.concatenate(loc_parts, axis=1)).astype(hdt),
            "vecs_tailT": vecs_tailT.astype(hdt),
            "wmat": wmat.astype(hdt),
        }
        in_maps.append(im)
    return in_maps


def kernel(vectors, Wl, Wr, branching, n_levels, _mode="fp16"):
    _lazy_imports()
    assert int(branching) == B and int(n_levels) == L
    vectors = np.asarray(vectors)
    assert vectors.shape == (int(OFFSETS[L]), D), vectors.shape

    nc = _build_nc(mode=_mode)
    in_maps = _prep_inputs(vectors, Wl, Wr, mode=_mode)
    try:
        res = run_bass_kernel_spmd(nc, in_maps, core_ids=list(range(N_CORES)),
                                   trace=False)
    except Exception:
        # transient device hiccups (e.g. NRT_EXEC_UNIT_UNRECOVERABLE right
        # after another process released the cores) clear on a retry
        res = run_bass_kernel_spmd(nc, in_maps, core_ids=list(range(N_CORES)),
                                   trace=False)
    root = res.results[0]["out"]
    # undo the on-chip transposed layout: [128 k, 2 mh] -> d = mh*128 + k
    return np.asarray(root).astype(np.float32).T.reshape(1, D)
